# revision 5
# baseline (speedup 1.0000x reference)
"""Trainium2 Bass kernel for nn_Attention2d (sparse_attention) — v2.

Reference (B=1): qkv = x @ Wqkv.T + bq; per (s,h): P = softmax_j(q.k^T +
mask); o = (P*bias) @ v; out = o @ Wout.T + bo.

Sharding: data-parallel over S (4 rows/core, 8 cores), no collectives.

Cost-model-driven design (TimelineSim is the scored metric):
  * matmul cost = out-free-size x pe_cycle x dtype-rate. So the o-matmul is
    FLIPPED: stationary = P^T-block [j,128i], moving = v [j,32] -> out free
    32 (4x cheaper than the [d,384i] orientation). Softmax denominators are
    flipped likewise (moving = masked-ones column, out [128i,1] ~ free).
  * q/k projection uses fp8e4 DoubleRow (K=256 in one step, 0.5 rate).
  * All host->device tensors are shipped PRE-TRANSPOSED / packed so no DMA
    transposes are needed; output is bf16 (host upcasts).
  * exp is the biggest fixed cost (ACT ~0.83ns/elem): part of the (s,h)
    tiles use a Schraudolph bit-trick instead (q pre-scaled by A=128/ln2 in
    the qkT copy, so pt_bits = u16(logit + B) is ONE DVE/Pool op), engine
    split tunable via EXP_ENG/PBT_ENG knobs.
  * mask is applied by zeroing masked v rows at the v PSUM->SBUF copy and
    via the masked-ones den column; P itself is never masked.
  * rec = 1/den broadcast [128,8,32]; o = po * rec fused into the PSUM->SBUF
    copy; o transposed via PE (identity shipped) into a spare PSUM region.
"""

import ml_dtypes
import numpy as np

import concourse.bass as bass
import concourse.tile as tile
import concourse.mybir as mybir
from concourse import bacc
from concourse.bass_utils import run_bass_kernel_spmd

B, S, R, D = 1, 32, 384, 256
H, HD = 8, 32
NCORES = 8
SS = S // NCORES          # 4 sequence rows per core
M = SS * R                # 1536 rows per core
MT = M // 128             # 12
JT = R // 128             # 3
KT = D // 128             # 2
F32 = mybir.dt.float32
BF16 = mybir.dt.bfloat16
FP8 = mybir.dt.float8e4
U16 = mybir.dt.uint16
AF = mybir.ActivationFunctionType
ALU = mybir.AluOpType
PM = mybir.MatmulPerfMode

# Schraudolph constants (bf16 bit trick): pt_bits = u16(A*logit + BS)
SCH_A = 128.0 / np.log(2.0)          # folded into the q-copy scale
SCH_B = 127.0 * 128.0 - 4.0          # minimax offset (~+-3.5% rel err)

# ---- engine split knobs -------------------------------------------------
# exp engine per (s,h) tile: "A"=ACT exp, "D"=DVE schraudolph, "P"=Pool
EXP_ENG = ["A"] * 32
# pbt engine per (s,h): "D"=DVE, "P"=Pool
PBT_ENG = (["D"] * 7 + ["P"]) * 4
QK_COPY_ENG = ["A", "D"] * 6          # 12 qkT psum->sbuf copies
V_COPY_ENG = ["A", "D"] * 6           # 12 masked v copies
OT_COPY_ENG = ["D", "A"] * 6          # 12 oT psum->sbuf copies
FO_COPY_ENG = ["D", "A"] * 6          # 12 out-proj copies (PSUM: no Pool)
ODIV_ENG = ["D"] * 12                 # 12 po*rec copies

# bf16 payload offsets
OB_XT = 0                                  # [128,2,M]
OB_WVWO = OB_XT + 128 * 2 * M              # [128,2,1024] (wq | wv | wo)
OB_BIAS = OB_WVWO + 128 * 2 * 1024         # [3,128,8,384] bias^T
OB_ID = OB_BIAS + JT * 128 * H * R         # [128,128] identity
NB = OB_ID + 128 * 128
# fp32 payload offsets
OF_BQ = 0                                  # [128,6]  b_qkv (nt-major)
OF_M01 = OF_BQ = 0
OF_M01 = 128 * 6                           # [128,4,3] keep-mask
NF = OF_M01 + 128 * SS * JT
# generic-bias extras (separate small tensor, only when biases nonzero)
NG = 2 * D                                 # bv | bo as [2,256]


def build_program(zero_bias: bool = True) -> bass.Bass:
    nc = bacc.Bacc("TRN2", target_bir_lowering=False, debug=False,
                   num_devices=NCORES)
    ab = nc.dram_tensor("allin_bf", [NB], BF16, kind="ExternalInput")
    af = nc.dram_tensor("allin_f32", [NF], F32, kind="ExternalInput")
    ag = None
    if not zero_bias:
        ag = nc.dram_tensor("allin_gb", [NG], F32, kind="ExternalInput")
    out_dram = nc.dram_tensor("out", [M, D], BF16, kind="ExternalOutput")
    with tile.TileContext(nc) as tc:
        _emit(nc, tc, ab, af, ag, out_dram, zero_bias)
    nc.compile()
    return nc


def _emit(nc, tc, ab, af, ag, out_dram, zero_bias):
    from contextlib import ExitStack
    ctx = ExitStack()
    with ctx:
        sg = ctx.enter_context(tc.tile_pool(name="sg", bufs=1))

        # ---- DMAs (sync engine; order = consumer order) ----
        f32s = sg.tile([128, 6 + SS * JT], F32)
        nc.sync.dma_start(
            out=f32s[:],
            in_=af[:].rearrange("(p c) -> p c", p=128))
        xT = sg.tile([128, 2, M], BF16)
        nc.sync.dma_start(
            out=xT[:],
            in_=ab[OB_XT:OB_WVWO].rearrange("(p k m) -> p k m", p=128, k=2))
        wvwo = sg.tile([128, 2, 1024], BF16)
        nc.sync.dma_start(
            out=wvwo[:],
            in_=ab[OB_WVWO:OB_BIAS].rearrange("(p k n) -> p k n", p=128, k=2))
        ident = sg.tile([128, 128], BF16)
        nc.sync.dma_start(
            out=ident[:],
            in_=ab[OB_ID:NB].rearrange("(p c) -> p c", p=128))
        biasT = sg.tile([128, JT, H, R], BF16)
        bias_src = ab[OB_BIAS:OB_ID].rearrange(
            "(jt p h i) -> p jt h i", jt=JT, p=128, h=H)
        for hp in range(4):
            nc.sync.dma_start(out=biasT[:, :, 2 * hp:2 * hp + 2, :],
                              in_=bias_src[:, :, 2 * hp:2 * hp + 2, :])
        if not zero_bias:
            gbs = sg.tile([2, D], F32)
            nc.sync.dma_start(
                out=gbs[:], in_=ag[:].rearrange("(a b) -> a b", a=2))

        bq = f32s[:, 0:6]
        m01f = f32s[:, 6:6 + SS * JT].rearrange("p (s j) -> p s j", s=SS)
        m01b = sg.tile([128, SS, JT], BF16)
        nc.vector.tensor_copy(m01b[:], m01f)
        if not zero_bias:
            bv_bf = sg.tile([1, D], BF16)
            nc.vector.tensor_copy(bv_bf[:], gbs[0:1, :])
            bo_bf = sg.tile([1, D], BF16)
            nc.vector.tensor_copy(bo_bf[:], gbs[1:2, :])
            ones_k1 = sg.tile([1, 128], BF16)
            nc.vector.memset(ones_k1[:], 1.0)

        qkT = sg.tile([128, 4, M], BF16)
        vsb = sg.tile([128, MT, D], BF16)
        o_sb = sg.tile([128, MT, D], BF16)
        oT = sg.tile([128, KT, M], BF16)
        fo = sg.tile([128, MT, D], BF16)

        # ---- Phase 1: projections ----
        with tc.tile_pool(name="psp", bufs=4, space="PSUM") as psp:
            ci = 0
            for nt in range(4):
                for mc in range(3):
                    pqk = psp.tile([128, 512], F32, tag="pqk")
                    for kt in range(KT):
                        nc.tensor.matmul(
                            pqk[:], wvwo[:, kt, nt * 128:(nt + 1) * 128],
                            xT[:, kt, mc * 512:(mc + 1) * 512],
                            start=(kt == 0), stop=(kt == KT - 1))
                    dst = qkT[:, nt, mc * 512:(mc + 1) * 512]
                    e = QK_COPY_ENG[ci]
                    if zero_bias:
                        if e == "A":
                            nc.scalar.copy(dst, pqk[:])
                        elif e == "D":
                            nc.vector.tensor_copy(dst, pqk[:])
                        else:
                            nc.gpsimd.tensor_copy(dst, pqk[:])
                    else:
                        eng2 = nc.vector if e == "D" else nc.gpsimd
                        eng2.tensor_scalar_add(dst, pqk[:], bq[:, nt:nt + 1])
                    ci += 1
            ci = 0
            for mt in range(MT):
                s, jt = mt // JT, mt % JT
                pv = psp.tile([128, D], F32, tag="pv")
                if not zero_bias:
                    nc.tensor.matmul(pv[:], ones_k1[:], bv_bf[:],
                                     start=True, stop=False)
                for kt in range(KT):
                    nc.tensor.matmul(
                        pv[:], xT[:, kt, mt * 128:(mt + 1) * 128],
                        wvwo[:, kt, 512:512 + D],
                        start=(zero_bias and kt == 0), stop=(kt == KT - 1))
                # masked v: multiply by keep-mask while leaving PSUM
                if V_COPY_ENG[ci] == "A":
                    nc.scalar.activation(vsb[:, mt, :], pv[:], AF.Copy,
                                         scale=m01f[:, s, jt:jt + 1])
                else:
                    nc.vector.tensor_scalar_mul(vsb[:, mt, :], pv[:],
                                                m01f[:, s, jt:jt + 1])
                ci += 1

        # ---- Phase 2: attention ----
        pt_pool = ctx.enter_context(tc.tile_pool(name="ptp", bufs=10))
        pbt_pool = ctx.enter_context(tc.tile_pool(name="pbtp", bufs=10))
        rec_pool = ctx.enter_context(tc.tile_pool(name="recp", bufs=2))
        with tc.tile_pool(name="lg", bufs=2, space="PSUM") as lgp, \
             tc.tile_pool(name="pod", bufs=2, space="PSUM") as podp:
            for s in range(SS):
                pts = []
                pbts = []
                for h in range(H):
                    g, hp = h // 4, h % 4
                    lg = lgp.tile([128, JT, 512], F32, tag="lg")
                    for jt in range(JT):
                        nc.tensor.matmul(
                            lg[:, jt, 0:R],
                            qkT[32 * hp:32 * hp + 32, 2 + g,
                                s * R + jt * 128:s * R + (jt + 1) * 128],
                            qkT[32 * hp:32 * hp + 32, g, s * R:(s + 1) * R],
                            start=True, stop=True,
                            tile_position=(32 * hp, 0))
                    pt = pt_pool.tile([128, JT, R], BF16, tag="pt")
                    ee = EXP_ENG[s * 8 + h]
                    if ee == "A":
                        nc.scalar.activation(pt[:], lg[:, :, 0:R], AF.Exp)
                    else:
                        eng = nc.vector if ee == "D" else nc.gpsimd
                        eng.tensor_scalar(pt[:].bitcast(U16), lg[:, :, 0:R],
                                          float(SCH_B), None, ALU.add)
                    pts.append(pt)
                    pbt = pbt_pool.tile([128, JT, R], BF16, tag="pbt")
                    if PBT_ENG[s * 8 + h] == "D":
                        nc.vector.tensor_tensor(pbt[:], pt[:],
                                                biasT[:, :, h, :], ALU.mult)
                    else:
                        nc.gpsimd.tensor_tensor(pbt[:], pt[:],
                                                biasT[:, :, h, :], ALU.mult)
                    pbts.append(pbt)
                for it in range(JT):
                    pod = podp.tile([128, 512], F32, tag="pod")
                    ib = slice(it * 128, (it + 1) * 128)
                    for h in range(H):
                        for jt in range(JT):
                            nc.tensor.matmul(
                                pod[:, 32 * h:32 * h + 32],
                                pbts[h][:, jt, ib],
                                vsb[:, 3 * s + jt, 32 * h:32 * h + 32],
                                start=(jt == 0), stop=(jt == JT - 1))
                        for jt in range(JT):
                            nc.tensor.matmul(
                                pod[:, 256 + h:257 + h],
                                pts[h][:, jt, ib],
                                m01b[:, s, jt:jt + 1],
                                start=(jt == 0), stop=(jt == JT - 1))
                    rec = rec_pool.tile([128, H, HD], F32, tag="rec")
                    nc.vector.reciprocal(
                        rec[:],
                        pod[:, 256:264][:, :, None].broadcast_to(
                            [128, H, HD]))
                    mt = 3 * s + it
                    if ODIV_ENG[mt] == "D":
                        nc.vector.tensor_tensor(
                            o_sb[:, mt, :], pod[:, 0:256],
                            rec[:].rearrange("p a b -> p (a b)"), ALU.mult)
                    else:
                        nc.gpsimd.tensor_tensor(
                            o_sb[:, mt, :], pod[:, 0:256],
                            rec[:].rearrange("p a b -> p (a b)"), ALU.mult)
                    # transpose o into spare region of the same PSUM bank
                    ptr = pod[:, 280:408].bitcast(BF16)  # [128,256] bf16
                    for half in range(KT):
                        nc.tensor.transpose(
                            ptr[:, half * 128:(half + 1) * 128],
                            o_sb[:, mt, half * 128:(half + 1) * 128],
                            ident[:])
                    dst = oT[:, :, mt * 128:(mt + 1) * 128]
                    if OT_COPY_ENG[mt] == "D":
                        nc.vector.tensor_copy(
                            dst, ptr[:].rearrange("p (k c) -> p k c", k=KT))
                    else:
                        nc.scalar.copy(
                            dst, ptr[:].rearrange("p (k c) -> p k c", k=KT))
                # out-projection for this s
                for mt in range(3 * s, 3 * s + 3):
                    pf = podp.tile([128, 512], F32, tag="pod")
                    if not zero_bias:
                        nc.tensor.matmul(pf[:, 0:D], ones_k1[:], bo_bf[:],
                                         start=True, stop=False)
                    for kt in range(KT):
                        nc.tensor.matmul(
                            pf[:, 0:D], oT[:, kt, mt * 128:(mt + 1) * 128],
                            wvwo[:, kt, 768:768 + D],
                            start=(zero_bias and kt == 0),
                            stop=(kt == KT - 1))
                    if FO_COPY_ENG[mt] == "A":
                        nc.scalar.copy(fo[:, mt, :], pf[:, 0:D])
                    elif FO_COPY_ENG[mt] == "D":
                        nc.vector.tensor_copy(fo[:, mt, :], pf[:, 0:D])
                    else:
                        nc.gpsimd.tensor_copy(fo[:, mt, :], pf[:, 0:D])
                nc.sync.dma_start(
                    out=out_dram[s * R:(s + 1) * R, :].rearrange(
                        "(a p) d -> p a d", p=128),
                    in_=fo[:, 3 * s:3 * s + 3, :])


def make_in_maps(pair_act, attention_mask, bias, W_qkv, b_qkv, W_out, b_out):
    bf = ml_dtypes.bfloat16
    f8 = ml_dtypes.float8_e4m3
    pair = np.asarray(pair_act, np.float32)[0]          # [S,R,D]
    Wq = np.asarray(W_qkv, np.float32)                  # [768,256]
    Wo = np.asarray(W_out, np.float32)                  # [256,256]
    biasf = np.asarray(bias, np.float32)[0, 0]          # [H,R,R]
    mask01 = 1.0 - np.asarray(attention_mask, np.float32)[0]  # [S,R] keep

    # shared pieces
    wqk = Wq[0:512].T.reshape(2, 128, 512)              # (kt,p,n)
    wv = Wq[512:768].T.reshape(2, 128, D)               # (kt,p,dv)
    wo = Wo.T.reshape(2, 128, D)
    wvwo = np.ascontiguousarray(
        np.concatenate([wqk, wv, wo], axis=2).transpose(1, 0, 2)).astype(bf)
    bias_t = np.ascontiguousarray(
        biasf.transpose(2, 0, 1)                        # [j,h,i]
        .reshape(JT, 128, H, R)).astype(bf)
    identity = np.eye(128, dtype=np.float32).astype(bf)
    shared_bf = np.concatenate(
        [wvwo.ravel(), bias_t.ravel(), identity.ravel()])
    bq6 = np.zeros((6, 128), np.float32)
    bq6.reshape(-1)[0:768] = np.asarray(b_qkv, np.float32)
    bq6 = bq6.T                                          # [128,6] nt-major

    gb = np.concatenate([np.asarray(b_qkv, np.float32)[512:768],
                         np.asarray(b_out, np.float32)])

    in_maps = []
    for c in range(NCORES):
        x = pair[c * SS:(c + 1) * SS].reshape(M, D)
        xT = np.ascontiguousarray(x.T.reshape(2, 128, M).transpose(1, 0, 2))
        abf = np.concatenate([xT.astype(bf).ravel(), shared_bf])
        m01 = np.ascontiguousarray(
            mask01[c * SS:(c + 1) * SS].reshape(SS, JT, 128)
            .transpose(2, 0, 1))                         # [128,s,jt]
        afl = np.concatenate([bq6, m01.reshape(128, SS * JT)],
                             axis=1).ravel()             # [128,18] row-major
        assert abf.size == NB and afl.size == NF
        m = {"allin_bf": np.ascontiguousarray(abf.astype(bf)),
             "allin_f32": np.ascontiguousarray(afl.astype(np.float32))}
        zb = bool(np.all(np.asarray(b_qkv) == 0)
                  and np.all(np.asarray(b_out) == 0))
        if not zb:
            m["allin_gb"] = np.ascontiguousarray(gb)
        in_maps.append(m)
    return in_maps


_PROGRAM_CACHE = {}


def kernel(pair_act, attention_mask, bias, W_qkv, b_qkv, W_out, b_out,
           _want_results=False, **extra):
    in_maps = make_in_maps(pair_act, attention_mask, bias, W_qkv, b_qkv,
                           W_out, b_out)
    zero_bias = bool(np.all(np.asarray(b_qkv) == 0)
                     and np.all(np.asarray(b_out) == 0))
    key = ("nc", zero_bias)
    if key not in _PROGRAM_CACHE:
        _PROGRAM_CACHE[key] = build_program(zero_bias)
    nc = _PROGRAM_CACHE[key]
    res = run_bass_kernel_spmd(nc, in_maps, core_ids=list(range(NCORES)))
    out = np.concatenate(
        [np.asarray(r["out"], dtype=np.float32).reshape(SS, R, D)
         for r in res.results], axis=0)
    out = out.reshape(B, S, R, D)
    if _want_results:
        return out, res
    return out
